# revision 18
# baseline (speedup 1.0000x reference)
"""Trainium2 Bass kernel for nn_DeepModel_multi_12945031430869.

Computes, for heads h in 0..31:
    y[:, h] = relu(x @ W1[h] + b1[h]) @ W2[h] + b2[h]
    out[:, h*513:(h+1)*513] = [x, y[:, h]]          # [4096, 16416]

Sharding: head-parallel across 8 NeuronCores (4 heads per core). Each core
produces its own [4096, 4*513] column block; the host concatenates them and
inserts the y columns.

Per-core device program (final):
  - GEMM1 in bf16 on the PE array (fp32r measured ~2x slower per matmul on
    this HW). Per (head, row-tile) two [128, 1024] PSUM half-tiles (2 banks
    each, pool bufs=4 -> all 8 banks, 4 halves in flight) are filled by 16
    [128,128]x[128,512] matmuls, k-outer so each stationary x-block feeds
    4 matmuls. Steady-state issue measured at the warm roofline
    (~216-221 ns per 512-wide matmul, 2.4 GHz).
  - Epilogue folds |w2| into W1 columns (host), sorted by descending w2:
    cols [0,960) all-positive folded, [960,1088) mixed "M" region raw,
    [1088,2048) all-negative folded. Three DVE ops per (head, rt):
      bigA: (ps_a * +1) max sb1 -> sc[:, 0:1024],   accum -> accA
      bigB: (ps_b * -1) min sb1 -> sc[:, 1024:2048], accum -> accB
      M2:   sc[:, 960:1088] * v -> accum accM,  v = w2-1 (Ma) | -(w2+1) (Mb)
    so relu + GEMM2 collapse into ~1.2 passes over h on the Vector engine.
    Constant residues sum to b2eff = b2 + sum_f w2_f b1_f; the Scalar
    engine applies it (activation accum) writing y into a per-head
    [128, 32] staging tile -> one DMA per head to the compact y_d output.
    Host scatters y into the out columns.
  - Output x-blocks never touch the compute pipeline: they are staged
    DRAM->SBUF->DRAM on the GpSimd engine's SW-DGE queue (its ring
    backpressure cannot stall a compute engine FIFO), spread over heads
    0-2. Keeping them on the Sync/Scalar queues serialized the whole
    pipeline (10-40 us waits); DRAM->DRAM direct DMA ran at 25 GB/s.
  - Startup-latency-ordered preamble: only xT cols 0:1024, head-0 W1 and
    head-0 sb1 precede the first matmul; the rest streams in behind it.
    Next head's W1 is prefetched during rt==0 of the current head.
"""

import numpy as np

N = 4096
D_IN = 512
D_H = 2048
USED = 32
NCORES = 8
HPC = USED // NCORES  # heads per core = 4
KT = D_IN // 128      # k tiles = 4
RT = N // 128         # row tiles = 32
HB = D_H // 2         # psum half width = 1024

_PROGS = {}


def _build(mlo, mhi):
    import concourse.tile as tile
    import concourse.mybir as mybir
    from concourse import bacc

    f32 = mybir.dt.float32
    bf16 = mybir.dt.bfloat16
    mw = mhi - mlo

    nc = bacc.Bacc("TRN2", target_bir_lowering=False, debug=False)

    xT_d = nc.dram_tensor("xT", [KT, 128, N], bf16, kind="ExternalInput").ap()
    x_d = nc.dram_tensor("x", [N, D_IN], f32, kind="ExternalInput").ap()
    w1_d = nc.dram_tensor("w1", [HPC, 128, KT * D_H], bf16, kind="ExternalInput").ap()
    sb1_d = nc.dram_tensor("sb1", [128, HPC * D_H], bf16, kind="ExternalInput").ap()
    w2m_d = nc.dram_tensor("w2m", [128, HPC * mw], f32, kind="ExternalInput").ap()
    b2_d = nc.dram_tensor("b2r", [128, HPC], f32, kind="ExternalInput").ap()
    out_d = nc.dram_tensor("out", [N, HPC * 513], f32, kind="ExternalOutput").ap()
    y_d = nc.dram_tensor("yst", [HPC, 128, RT], f32, kind="ExternalOutput").ap()

    with tile.TileContext(nc) as tc:
        with tc.tile_pool(name="xt", bufs=1) as xtp, \
             tc.tile_pool(name="cst", bufs=1) as cst, \
             tc.tile_pool(name="w1p", bufs=2) as w1p, \
             tc.tile_pool(name="ys", bufs=2) as ysp, \
             tc.tile_pool(name="xc", bufs=4) as xcp, \
             tc.tile_pool(name="ps", bufs=4, space="PSUM") as pp, \
             tc.tile_pool(name="scr", bufs=3) as scr, \
             tc.tile_pool(name="sm", bufs=3) as smp, \
             tc.tile_pool(name="acc", bufs=6) as accp, \
             tc.tile_pool(name="j3", bufs=6) as j3p:

            # startup-latency-ordered preamble: only what the first row
            # tiles touch (xT cols 0:1024 = rt 0..7, w1 head 0, sb1 head 0)
            # is transferred before the first matmul can fire
            # dual-queue preamble: xT rides the Sync HWDGE queue while W1
            # head 0 + head-0 sb1 + consts ride the Scalar HWDGE queue in
            # parallel, so the first matmul fires after ~max(1MB, 2.5MB)
            # instead of their sum
            xts = []
            w1ts = [w1p.tile([128, KT * D_H], bf16, tag="w1", name="w1t0")]
            nc.scalar.dma_start(w1ts[0][:, 0:D_H], w1_d[0, :, 0:D_H])
            for k in range(KT):
                t = xtp.tile([128, N], bf16, tag=f"x{k}")
                nc.sync.dma_start(t[:, 0:1024], xT_d[k, :, 0:1024])
                xts.append(t)
            nc.scalar.dma_start(w1ts[0][:, D_H:], w1_d[0, :, D_H:])
            sb1 = cst.tile([128, HPC * D_H], bf16, tag="sb1")
            nc.scalar.dma_start(sb1[:, 0:D_H], sb1_d[:, 0:D_H])
            w2m = cst.tile([128, HPC * mw], f32, tag="w2m")
            nc.scalar.dma_start(w2m[:], w2m_d[:])
            b2r = cst.tile([128, HPC], f32, tag="b2r")
            nc.scalar.dma_start(b2r[:], b2_d[:])
            for k in range(KT):
                nc.sync.dma_start(xts[k][:, 1024:N], xT_d[k, :, 1024:N])
            nc.sync.dma_start(sb1[:, D_H:], sb1_d[:, D_H:])

            mx = mybir.AluOpType.max
            mn = mybir.AluOpType.min
            mult = mybir.AluOpType.mult
            ident = mybir.ActivationFunctionType.Identity

            for h in range(HPC):
                w1t = w1ts[h]
                ys = ysp.tile([128, RT], f32, tag="ys", name=f"ys{h}")
                for rt in range(RT):
                    rs = rt * 128
                    if h == min(rt // 11, 2):
                        # x-block copies staged through SBUF, issued from the
                        # GpSimd engine (SW DGE) so DGE ring backpressure
                        # never stalls a compute engine's FIFO; spread across
                        # heads. DRAM->DRAM direct was 14x slower (25 GB/s).
                        xst = xcp.tile([128, 512], f32, tag="xst")
                        nc.gpsimd.dma_start(xst[:], x_d[rs:rs + 128, :])
                        for hh in range(HPC):
                            nc.gpsimd.dma_start(
                                out_d[rs:rs + 128, hh * 513:hh * 513 + 512],
                                xst[:],
                            )
                    ps_a = pp.tile([128, HB], f32, tag="ps", name=f"psa{h}_{rt}")
                    ps_b = pp.tile([128, HB], f32, tag="ps", name=f"psb{h}_{rt}")
                    for k in range(KT):
                        for tt in range(4):
                            ps = ps_a if tt < 2 else ps_b
                            nc.tensor.matmul(
                                ps[:, (tt % 2) * 512:(tt % 2) * 512 + 512],
                                lhsT=xts[k][:, rs:rs + 128],
                                rhs=w1t[:, k * D_H + tt * 512:k * D_H + (tt + 1) * 512],
                                start=(k == 0),
                                stop=(k == KT - 1),
                            )
                    if rt == 0 and h + 1 < HPC:
                        w1ts.append(w1p.tile([128, KT * D_H], bf16, tag="w1",
                                             name=f"w1t{h + 1}"))
                        nc.sync.dma_start(w1ts[h + 1][:], w1_d[h + 1])
                    c0 = h * D_H
                    acc = accp.tile([128, 3], f32, tag="acc")
                    sc = scr.tile([128, D_H], f32, tag="sc")
                    nc.vector.scalar_tensor_tensor(
                        out=sc[:, 0:HB],
                        in0=ps_a[:],
                        scalar=1.0,
                        in1=sb1[:, c0:c0 + HB],
                        op0=mult,
                        op1=mx,
                        accum_out=acc[:, 0:1],
                    )
                    nc.vector.scalar_tensor_tensor(
                        out=sc[:, HB:D_H],
                        in0=ps_b[:],
                        scalar=-1.0,
                        in1=sb1[:, c0 + HB:c0 + D_H],
                        op0=mult,
                        op1=mn,
                        accum_out=acc[:, 1:2],
                    )
                    sm = smp.tile([128, mw], f32, tag="sm")
                    nc.vector.scalar_tensor_tensor(
                        out=sm[:],
                        in0=sc[:, mlo:mhi],
                        scalar=1.0,
                        in1=w2m[:, h * mw:(h + 1) * mw],
                        op0=mult,
                        op1=mult,
                        accum_out=acc[:, 2:3],
                    )
                    j3 = j3p.tile([128, 3], f32, tag="j3")
                    nc.scalar.activation(
                        j3[:], acc[:], ident,
                        bias=b2r[:, h:h + 1], scale=1.0,
                        accum_out=ys[:, rt:rt + 1],
                    )
                nc.sync.dma_start(y_d[h], ys[:])

    nc.compile()
    return nc


def _get_program(mlo, mhi):
    key = (mlo, mhi)
    if key not in _PROGS:
        _PROGS[key] = _build(mlo, mhi)
    return _PROGS[key]


def kernel(x, W1, b1, W2, b2):
    import ml_dtypes
    from concourse.bass_utils import run_bass_kernel_spmd

    x = np.asarray(x, dtype=np.float32)
    W1 = np.asarray(W1, dtype=np.float32)
    b1 = np.asarray(b1, dtype=np.float32)
    W2 = np.asarray(W2, dtype=np.float32)
    b2 = np.asarray(b2, dtype=np.float32)

    # M region must contain every head's sign boundary; widen if needed
    P = (W2[:USED] > 0).sum(axis=1)
    mlo, mhi = 960, 1088
    if P.min() < mlo:
        mlo = max(0, int(P.min()) // 64 * 64)
    if P.max() > mhi:
        mhi = min(D_H, -(-int(P.max()) // 64) * 64)
    assert mlo < 1024 < mhi
    mw = mhi - mlo

    nc = _get_program(mlo, mhi)

    xT = np.ascontiguousarray(x.T).astype(ml_dtypes.bfloat16).reshape(KT, 128, N)

    in_maps = []
    for c in range(NCORES):
        w1heads = []
        sb1cols = np.empty(HPC * D_H, dtype=np.float32)
        w2mcols = np.empty(HPC * mw, dtype=np.float32)
        b2eff = np.empty(HPC, dtype=np.float32)
        for i in range(HPC):
            h = HPC * c + i
            w2 = W2[h]
            order = np.argsort(-w2, kind="stable")  # descending w2
            w2s = w2[order]
            b1s = b1[h][order]
            if mlo > 0:
                assert w2s[mlo - 1] > 0, "region overflow (A)"
            if mhi < D_H:
                assert w2s[mhi] < 0, "region overflow (B)"
            aw = np.abs(w2s)
            scale = aw.copy()
            scale[mlo:mhi] = 1.0  # M region stays unfolded
            w1s = W1[h][:, order] * scale[None, :]
            sb = np.empty(D_H, dtype=np.float32)
            sb[:mlo] = -aw[:mlo] * b1s[:mlo]
            sb[mlo:1024] = -b1s[mlo:1024]          # Ma (raw, in bigA)
            sb[1024:mhi] = b1s[1024:mhi]           # Mb (raw, in bigB)
            sb[mhi:] = aw[mhi:] * b1s[mhi:]
            sb1cols[i * D_H:(i + 1) * D_H] = sb
            w2mcols[i * mw:i * mw + (1024 - mlo)] = w2s[mlo:1024] - 1.0
            w2mcols[i * mw + (1024 - mlo):(i + 1) * mw] = -(w2s[1024:mhi] + 1.0)
            b2eff[i] = (
                b2[h].astype(np.float64)
                + np.dot(w2.astype(np.float64), b1[h].astype(np.float64))
            ) / 3.0
            # [512, 2048] -> [128 part, KT*D_H] with cols = k*D_H + j
            w1heads.append(
                np.ascontiguousarray(
                    w1s.reshape(KT, 128, D_H).transpose(1, 0, 2)
                ).reshape(128, KT * D_H)
            )
        in_maps.append({
            "xT": xT,
            "x": x,
            "w1": np.ascontiguousarray(
                np.stack(w1heads, axis=0).astype(ml_dtypes.bfloat16)
            ),
            "sb1": np.ascontiguousarray(
                np.broadcast_to(
                    sb1cols.reshape(1, -1).astype(ml_dtypes.bfloat16),
                    (128, HPC * D_H),
                )
            ),
            "w2m": np.ascontiguousarray(
                np.broadcast_to(w2mcols.reshape(1, -1), (128, HPC * mw))
            ),
            "b2r": np.ascontiguousarray(
                np.broadcast_to(b2eff.reshape(1, -1), (128, HPC))
            ),
        })

    import os
    trace = os.environ.get("BASS_KERNEL_TRACE") == "1"
    if trace:
        import sys
        sys.path.insert(0, "/tmp")
        try:
            import axon_shim
            axon_shim.install()
        except Exception:
            trace = False
    res = run_bass_kernel_spmd(nc, in_maps, list(range(NCORES)), trace=trace)
    kernel.last_result = res

    out = np.concatenate([res.results[c]["out"] for c in range(NCORES)], axis=1)
    for c in range(NCORES):
        yst = res.results[c]["yst"]  # [HPC, 128, RT]
        for i in range(HPC):
            # y[n] for n = rt*128 + p  lives at yst[i, p, rt]
            out[:, (c * HPC + i) * 513 + 512] = yst[i].T.reshape(N)
    return out


# revision 19
# speedup vs baseline: 1.0168x; 1.0168x over previous
"""Trainium2 Bass kernel for nn_DeepModel_multi_12945031430869.

Computes, for heads h in 0..31:
    y[:, h] = relu(x @ W1[h] + b1[h]) @ W2[h] + b2[h]
    out[:, h*513:(h+1)*513] = [x, y[:, h]]          # [4096, 16416]

Sharding: head-parallel across 8 NeuronCores (4 heads per core). Each core
produces its own [4096, 4*513] column block; the host concatenates them and
inserts the y columns.

Per-core device program (final):
  - GEMM1 in bf16 on the PE array (fp32r measured ~2x slower per matmul on
    this HW). Per (head, row-tile) two [128, 1024] PSUM half-tiles (2 banks
    each, pool bufs=4 -> all 8 banks, 4 halves in flight) are filled by 16
    [128,128]x[128,512] matmuls, k-outer so each stationary x-block feeds
    4 matmuls. Steady-state issue measured at the warm roofline
    (~216-221 ns per 512-wide matmul, 2.4 GHz).
  - Epilogue folds |w2| into W1 columns (host), sorted by descending w2:
    cols [0,960) all-positive folded, [960,1088) mixed "M" region raw,
    [1088,2048) all-negative folded. Three DVE ops per (head, rt):
      bigA: (ps_a * +1) max sb1 -> sc[:, 0:1024],   accum -> accA
      bigB: (ps_b * -1) min sb1 -> sc[:, 1024:2048], accum -> accB
      M2:   sc[:, 960:1088] * v -> accum accM,  v = w2-1 (Ma) | -(w2+1) (Mb)
    so relu + GEMM2 collapse into ~1.2 passes over h on the Vector engine.
    Constant residues sum to b2eff = b2 + sum_f w2_f b1_f; the Scalar
    engine applies it (activation accum) writing y into a per-head
    [128, 32] staging tile -> one DMA per head to the compact y_d output.
    Host scatters y into the out columns.
  - Output x-blocks never touch the compute pipeline: they are staged
    DRAM->SBUF->DRAM on the GpSimd engine's SW-DGE queue (its ring
    backpressure cannot stall a compute engine FIFO), spread over heads
    0-2. Keeping them on the Sync/Scalar queues serialized the whole
    pipeline (10-40 us waits); DRAM->DRAM direct DMA ran at 25 GB/s.
  - Startup-latency-ordered preamble: only xT cols 0:1024, head-0 W1 and
    head-0 sb1 precede the first matmul; the rest streams in behind it.
    Next head's W1 is prefetched during rt==0 of the current head.
"""

import numpy as np

N = 4096
D_IN = 512
D_H = 2048
USED = 32
NCORES = 8
HPC = USED // NCORES  # heads per core = 4
KT = D_IN // 128      # k tiles = 4
RT = N // 128         # row tiles = 32
HB = D_H // 2         # psum half width = 1024

_PROGS = {}


def _build(mlo, mhi):
    import concourse.tile as tile
    import concourse.mybir as mybir
    from concourse import bacc

    f32 = mybir.dt.float32
    bf16 = mybir.dt.bfloat16
    mw = mhi - mlo

    nc = bacc.Bacc("TRN2", target_bir_lowering=False, debug=False)

    xT_d = nc.dram_tensor("xT", [KT, 128, N], bf16, kind="ExternalInput").ap()
    x_d = nc.dram_tensor("x", [N, D_IN], f32, kind="ExternalInput").ap()
    w1_d = nc.dram_tensor("w1", [HPC, 128, KT * D_H], bf16, kind="ExternalInput").ap()
    sb1_d = nc.dram_tensor("sb1", [128, HPC * D_H], bf16, kind="ExternalInput").ap()
    w2m_d = nc.dram_tensor("w2m", [128, HPC * mw], f32, kind="ExternalInput").ap()
    b2_d = nc.dram_tensor("b2r", [128, HPC], f32, kind="ExternalInput").ap()
    out_d = nc.dram_tensor("out", [N, HPC * 513], f32, kind="ExternalOutput").ap()
    y_d = nc.dram_tensor("yst", [HPC, 128, RT], f32, kind="ExternalOutput").ap()

    with tile.TileContext(nc) as tc:
        with tc.tile_pool(name="xt", bufs=1) as xtp, \
             tc.tile_pool(name="cst", bufs=1) as cst, \
             tc.tile_pool(name="w1p", bufs=2) as w1p, \
             tc.tile_pool(name="ys", bufs=2) as ysp, \
             tc.tile_pool(name="xc", bufs=4) as xcp, \
             tc.tile_pool(name="ps", bufs=4, space="PSUM") as pp, \
             tc.tile_pool(name="scr", bufs=3) as scr, \
             tc.tile_pool(name="sm", bufs=3) as smp, \
             tc.tile_pool(name="acc", bufs=6) as accp, \
             tc.tile_pool(name="j3", bufs=6) as j3p:

            # startup-latency-ordered preamble: only what the first row
            # tiles touch (xT cols 0:1024 = rt 0..7, w1 head 0, sb1 head 0)
            # is transferred before the first matmul can fire
            # dual-queue preamble: xT rides the Sync HWDGE queue while W1
            # head 0 + head-0 sb1 + consts ride the Scalar HWDGE queue in
            # parallel, so the first matmul fires after ~max(1MB, 2.5MB)
            # instead of their sum
            xts = []
            w1ts = [w1p.tile([128, KT * D_H], bf16, tag="w1", name="w1t0")]
            nc.scalar.dma_start(w1ts[0][:], w1_d[0])
            for k in range(KT):
                t = xtp.tile([128, N], bf16, tag=f"x{k}")
                nc.sync.dma_start(t[:, 0:1024], xT_d[k, :, 0:1024])
                xts.append(t)
            sb1 = cst.tile([128, HPC * D_H], bf16, tag="sb1")
            nc.scalar.dma_start(sb1[:, 0:D_H], sb1_d[:, 0:D_H])
            w2m = cst.tile([128, HPC * mw], f32, tag="w2m")
            nc.scalar.dma_start(w2m[:], w2m_d[:])
            b2r = cst.tile([128, HPC], f32, tag="b2r")
            nc.scalar.dma_start(b2r[:], b2_d[:])
            for k in range(KT):
                nc.sync.dma_start(xts[k][:, 1024:N], xT_d[k, :, 1024:N])
            nc.sync.dma_start(sb1[:, D_H:], sb1_d[:, D_H:])

            mx = mybir.AluOpType.max
            mn = mybir.AluOpType.min
            mult = mybir.AluOpType.mult
            ident = mybir.ActivationFunctionType.Identity

            for h in range(HPC):
                w1t = w1ts[h]
                ys = ysp.tile([128, RT], f32, tag="ys", name=f"ys{h}")
                for rt in range(RT):
                    rs = rt * 128
                    if h == min(rt // 11, 2):
                        # x-block copies staged through SBUF, issued from the
                        # GpSimd engine (SW DGE) so DGE ring backpressure
                        # never stalls a compute engine's FIFO; spread across
                        # heads. DRAM->DRAM direct was 14x slower (25 GB/s).
                        xst = xcp.tile([128, 512], f32, tag="xst")
                        nc.gpsimd.dma_start(xst[:], x_d[rs:rs + 128, :])
                        for hh in range(HPC):
                            nc.gpsimd.dma_start(
                                out_d[rs:rs + 128, hh * 513:hh * 513 + 512],
                                xst[:],
                            )
                    ps_a = pp.tile([128, HB], f32, tag="ps", name=f"psa{h}_{rt}")
                    ps_b = pp.tile([128, HB], f32, tag="ps", name=f"psb{h}_{rt}")
                    for k in range(KT):
                        for tt in range(4):
                            ps = ps_a if tt < 2 else ps_b
                            nc.tensor.matmul(
                                ps[:, (tt % 2) * 512:(tt % 2) * 512 + 512],
                                lhsT=xts[k][:, rs:rs + 128],
                                rhs=w1t[:, k * D_H + tt * 512:k * D_H + (tt + 1) * 512],
                                start=(k == 0),
                                stop=(k == KT - 1),
                            )
                    if rt == 0 and h + 1 < HPC:
                        w1ts.append(w1p.tile([128, KT * D_H], bf16, tag="w1",
                                             name=f"w1t{h + 1}"))
                        nc.sync.dma_start(w1ts[h + 1][:], w1_d[h + 1])
                    c0 = h * D_H
                    acc = accp.tile([128, 3], f32, tag="acc")
                    sc = scr.tile([128, D_H], f32, tag="sc")
                    nc.vector.scalar_tensor_tensor(
                        out=sc[:, 0:HB],
                        in0=ps_a[:],
                        scalar=1.0,
                        in1=sb1[:, c0:c0 + HB],
                        op0=mult,
                        op1=mx,
                        accum_out=acc[:, 0:1],
                    )
                    nc.vector.scalar_tensor_tensor(
                        out=sc[:, HB:D_H],
                        in0=ps_b[:],
                        scalar=-1.0,
                        in1=sb1[:, c0 + HB:c0 + D_H],
                        op0=mult,
                        op1=mn,
                        accum_out=acc[:, 1:2],
                    )
                    sm = smp.tile([128, mw], f32, tag="sm")
                    nc.vector.scalar_tensor_tensor(
                        out=sm[:],
                        in0=sc[:, mlo:mhi],
                        scalar=1.0,
                        in1=w2m[:, h * mw:(h + 1) * mw],
                        op0=mult,
                        op1=mult,
                        accum_out=acc[:, 2:3],
                    )
                    j3 = j3p.tile([128, 3], f32, tag="j3")
                    nc.scalar.activation(
                        j3[:], acc[:], ident,
                        bias=b2r[:, h:h + 1], scale=1.0,
                        accum_out=ys[:, rt:rt + 1],
                    )
                nc.sync.dma_start(y_d[h], ys[:])

    nc.compile()
    return nc


def _get_program(mlo, mhi):
    key = (mlo, mhi)
    if key not in _PROGS:
        _PROGS[key] = _build(mlo, mhi)
    return _PROGS[key]


def kernel(x, W1, b1, W2, b2):
    import ml_dtypes
    from concourse.bass_utils import run_bass_kernel_spmd

    x = np.asarray(x, dtype=np.float32)
    W1 = np.asarray(W1, dtype=np.float32)
    b1 = np.asarray(b1, dtype=np.float32)
    W2 = np.asarray(W2, dtype=np.float32)
    b2 = np.asarray(b2, dtype=np.float32)

    # M region must contain every head's sign boundary; widen if needed
    P = (W2[:USED] > 0).sum(axis=1)
    mlo, mhi = 960, 1088
    if P.min() < mlo:
        mlo = max(0, int(P.min()) // 64 * 64)
    if P.max() > mhi:
        mhi = min(D_H, -(-int(P.max()) // 64) * 64)
    assert mlo < 1024 < mhi
    mw = mhi - mlo

    nc = _get_program(mlo, mhi)

    xT = np.ascontiguousarray(x.T).astype(ml_dtypes.bfloat16).reshape(KT, 128, N)

    in_maps = []
    for c in range(NCORES):
        w1heads = []
        sb1cols = np.empty(HPC * D_H, dtype=np.float32)
        w2mcols = np.empty(HPC * mw, dtype=np.float32)
        b2eff = np.empty(HPC, dtype=np.float32)
        for i in range(HPC):
            h = HPC * c + i
            w2 = W2[h]
            order = np.argsort(-w2, kind="stable")  # descending w2
            w2s = w2[order]
            b1s = b1[h][order]
            if mlo > 0:
                assert w2s[mlo - 1] > 0, "region overflow (A)"
            if mhi < D_H:
                assert w2s[mhi] < 0, "region overflow (B)"
            aw = np.abs(w2s)
            scale = aw.copy()
            scale[mlo:mhi] = 1.0  # M region stays unfolded
            w1s = W1[h][:, order] * scale[None, :]
            sb = np.empty(D_H, dtype=np.float32)
            sb[:mlo] = -aw[:mlo] * b1s[:mlo]
            sb[mlo:1024] = -b1s[mlo:1024]          # Ma (raw, in bigA)
            sb[1024:mhi] = b1s[1024:mhi]           # Mb (raw, in bigB)
            sb[mhi:] = aw[mhi:] * b1s[mhi:]
            sb1cols[i * D_H:(i + 1) * D_H] = sb
            w2mcols[i * mw:i * mw + (1024 - mlo)] = w2s[mlo:1024] - 1.0
            w2mcols[i * mw + (1024 - mlo):(i + 1) * mw] = -(w2s[1024:mhi] + 1.0)
            b2eff[i] = (
                b2[h].astype(np.float64)
                + np.dot(w2.astype(np.float64), b1[h].astype(np.float64))
            ) / 3.0
            # [512, 2048] -> [128 part, KT*D_H] with cols = k*D_H + j
            w1heads.append(
                np.ascontiguousarray(
                    w1s.reshape(KT, 128, D_H).transpose(1, 0, 2)
                ).reshape(128, KT * D_H)
            )
        in_maps.append({
            "xT": xT,
            "x": x,
            "w1": np.ascontiguousarray(
                np.stack(w1heads, axis=0).astype(ml_dtypes.bfloat16)
            ),
            "sb1": np.ascontiguousarray(
                np.broadcast_to(
                    sb1cols.reshape(1, -1).astype(ml_dtypes.bfloat16),
                    (128, HPC * D_H),
                )
            ),
            "w2m": np.ascontiguousarray(
                np.broadcast_to(w2mcols.reshape(1, -1), (128, HPC * mw))
            ),
            "b2r": np.ascontiguousarray(
                np.broadcast_to(b2eff.reshape(1, -1), (128, HPC))
            ),
        })

    import os
    trace = os.environ.get("BASS_KERNEL_TRACE") == "1"
    if trace:
        import sys
        sys.path.insert(0, "/tmp")
        try:
            import axon_shim
            axon_shim.install()
        except Exception:
            trace = False
    res = run_bass_kernel_spmd(nc, in_maps, list(range(NCORES)), trace=trace)
    kernel.last_result = res

    out = np.concatenate([res.results[c]["out"] for c in range(NCORES)], axis=1)
    for c in range(NCORES):
        yst = res.results[c]["yst"]  # [HPC, 128, RT]
        for i in range(HPC):
            # y[n] for n = rt*128 + p  lives at yst[i, p, rt]
            out[:, (c * HPC + i) * 513 + 512] = yst[i].T.reshape(N)
    return out
